# revision 30
# baseline (speedup 1.0000x reference)
"""Trainium2 Bass kernel for nn_CausalFeatureTransformer.

Only the label row of the reference output is needed, so the per-sample
transformer collapses to per-sample score maps plus head-wise weighted
sums.  All weight-only quantities (score columns, mask logs, UV
projections, folded affine/alpha constants) are computed on the host in
numpy; the device runs only the Z-dependent main phase:

  bn_stats on row-layout Z for per-sample mean/var, rank-1 PE matmuls
  to broadcast rows, s-chain on sqrt(vf)-prefolded transposed Z, scores
  via one double-broadcast tensor_tensor per chunk, exp with per-
  partition mask bias, attention numerators via UV-stationary matmuls
  and denominators via eT-stationary ones-matvecs (so the softmax
  reciprocal runs on a [128,4] column tile), final LN in [n,e] layout
  where mean/rstd are per-partition scalars (magic-constant Newton
  rsqrt on [128,1] columns), FFN with pre-scaled weights and rank-1
  bias matmuls.  No gpsimd ops (no library reloads); output is stored
  [e, n] and transposed on host.

Data-parallel over N: 1024 samples -> 8 cores x 128 samples.
"""
import numpy as np
from contextlib import ExitStack

import concourse.bass as bass
import concourse.tile as tile
from concourse import bacc, mybir
from concourse.bass_utils import run_bass_kernel_spmd

F32 = mybir.dt.float32
BF16 = mybir.dt.bfloat16
I32 = mybir.dt.int32
AF = mybir.ActivationFunctionType
OP = mybir.AluOpType

N, FD, E, H, DK, SEQ = 1024, 256, 128, 4, 32, 257
NCORES = 8
NP = N // NCORES
EPS = 1e-5
ISQ = float(1.0 / np.sqrt(DK))

# cearly (f32) column layout
C_ONE1 = 0        # 1.0 column
C_A2E = 1         # alpha^2*eps column
C_SQVF = 2        # sqrt(vf) chunk cols 2,3
C_MK = 4          # mask-log chunk cols 4,5
C_B1A = 6         # b1' halves cols 6,7
C_B1B = 7
C_IDENT = 8       # identity [128,128]
CE = 136
# cb (bf16) column layout
B_UV = 0          # UV chunks at 0:128, 128:256
B_BO4 = 256       # bo4 rows 0:4 at cols 256:384
B_CVB = 384       # cvbar row 0 at cols 384:512
B_W1 = 512        # diag(g2)@W1 at 512:768
B_W2A = 768       # alpha*W2[:128]
B_W2B = 896
B_ACOL = 1024     # acol chunks [H] at 1024:1028, 1028:1032
B_IDENT = 1032    # bf16 identity [128,128]
B_WO = 1160       # alpha*Wo bf16
B_WOBR = 1288     # wobar row 0
B_ULC = 1416      # ulc row 0 [1,128]
B_ECROW = 1544    # ec row 0 [1,4]
CB = 1548



def _body(tc, d, out_ap):
    nc = tc.nc
    ctx = ExitStack()
    with ctx:
        cp = ctx.enter_context(tc.tile_pool(name="cp", bufs=1))
        wp = ctx.enter_context(tc.tile_pool(name="wp", bufs=1))
        ps_att = ctx.enter_context(tc.tile_pool(name="ps_att", bufs=2, space="PSUM"))
        ps_mb = ctx.enter_context(tc.tile_pool(name="ps_mb", bufs=1, space="PSUM"))
        ps_sm = ctx.enter_context(tc.tile_pool(name="ps_sm", bufs=1, space="PSUM"))
        ps_big = ctx.enter_context(tc.tile_pool(name="ps_big", bufs=2, space="PSUM"))

        def sb(name, shape, pool=wp):
            return pool.tile(list(shape), F32, tag=name, name=name)

        # ---------------- loads ----------------
        # [z | cearly] in one transfer (both gate the front chain); zt after
        zin = sb("zin", [128, FD + CE], cp)
        nc.sync.dma_start(zin[:], d["zin"])
        z = zin[:, 0:FD]                        # row layout [n, f]
        ct = zin[:, FD:FD + CE]
        zt = sb("zt", [128, FD], cp)            # Z^T chunks [f, n]
        nc.sync.dma_start(zt[:], d["zt"])
        # big bf16 consts go out on the scalar queue, gated on zin completion,
        # so their descriptors don't round-robin-delay the zin transfer
        dumz = sb("dumz", [1, 1], cp)
        nc.scalar.copy(dumz[:], zin[0:1, 0:1])
        cba = cp.tile([128, CB + 2 * H * 128], BF16, tag="cba", name="cba")
        nc.scalar.dma_start(cba[:], d["cba"])
        cb = cba[:, 0:CB]
        acq = cba[:, CB:CB + 2 * H * 128].rearrange("p (a h b) -> p a h b",
                                                    a=2, h=H)

        one1 = ct[:, C_ONE1:C_ONE1 + 1]
        ident = ct[:, C_IDENT:C_IDENT + 128]
        identb = cb[:, B_IDENT:B_IDENT + 128]
        bo4b = cb[0:4, B_BO4:B_BO4 + 128]
        cvbarb = cb[0:1, B_CVB:B_CVB + 128]
        wobarb = cb[0:1, B_WOBR:B_WOBR + 128]
        ecrowb = cb[0:1, B_ECROW:B_ECROW + H]

        ones1 = sb("ones1", [1, 128], cp)
        nc.vector.memset(ones1[:], 1.0)
        oneb = cp.tile([128, 1], BF16, tag="oneb", name="oneb")
        nc.vector.memset(oneb[:], 1.0)
        onebr = cp.tile([1, 128], BF16, tag="onebr", name="onebr")
        nc.vector.memset(onebr[:], 1.0)

        # one bank: [0:128] m-bcast, [128:256] eps-bcast, [384:388] pz4
        p_mb = ps_mb.tile([128, 388], F32, tag="mb", name="p_mb")
        pz4 = p_mb[:, 384:388]
        p_att = ps_att.tile([128, 128], F32, tag="att", name="p_att")

        # ---------------- Z stats (row layout) + row broadcasts ----------------
        stZ = sb("stZ", [128, 6])
        nc.vector.bn_stats(stZ[:], z[:])
        me = sb("me", [128, 2])
        nc.vector.bn_aggr(me[:], stZ[:])
        nc.vector.tensor_scalar(out=me[:, 1:2], in0=me[:, 1:2], scalar1=EPS,
                                scalar2=EPS * EPS, op0=OP.mult, op1=OP.add)
        p_me = ps_sm.tile([1, 256], F32, tag="sm", name="p_me")
        nc.tensor.transpose(p_me[0:1, 0:128], me[:, 0:1], ident)
        nc.tensor.transpose(p_me[0:1, 128:256], me[:, 1:2], ident)
        mer = cp.tile([1, 256], BF16, tag="mer", name="mer")
        nc.vector.tensor_copy(out=mer[:, 0:128], in_=p_me[0:1, 0:128])
        nc.tensor.matmul(p_mb[:, 0:128], onebr[:], mer[:, 0:128], start=True,
                         stop=True, skip_group_check=True)
        nc.vector.tensor_copy(out=mer[:, 128:256], in_=p_me[0:1, 128:256])
        nc.tensor.matmul(p_mb[:, 128:256], onebr[:], mer[:, 128:256],
                         start=True, stop=True, skip_group_check=True)
        p_eps = p_mb[:, 128:256]
        # label-token rank-1 PSUM inits (after p_mb so it wins the PE queue)
        nc.tensor.matmul(pz4, onebr[:], ecrowb, start=True, stop=False,
                         skip_group_check=True)
        nc.tensor.matmul(p_att[:], onebr[:], cb[0:1, B_ULC:B_ULC + 128],
                         start=True, stop=False, skip_group_check=True)

        # ------ s, scores, exp, attention matmuls ------
        # two chunks, emitted stage-interleaved so chunk 1's vector stages
        # overlap chunk 0's scalar stages in the in-order engine queues
        cns, sqTs, w1ts, lnts, rsts, sTbs, scTs, eTs, wpres = \
            [], [], [], [], [], [], [], [], []
        for c in range(2):
            cns.append(sb(f"cn{c}", [128, 128]))
            sqTs.append(sb(f"sqT{c}", [128, 128]))
            w1ts.append(sb(f"w1t{c}", [128, 128]))
            lnts.append(sb(f"lnt{c}", [128, 128]))
            rsts.append(sb(f"rst{c}", [128, 128]))
            sTbs.append(wp.tile([128, 128], BF16, tag=f"sTb{c}", name=f"sTb{c}"))
            scTs.append(wp.tile([128, H, 128], BF16, tag=f"scT{c}",
                                name=f"scT{c}"))
            eTs.append(wp.tile([128, H, 128], BF16, tag=f"eT{c}", name=f"eT{c}"))
            wpres.append(wp.tile([128, H, 128], BF16, tag=f"wpre{c}",
                                 name=f"wpre{c}"))
        for c in range(2):
            nc.vector.tensor_tensor(out=cns[c][:],
                                    in0=zt[:, 128 * c:128 * (c + 1)],
                                    in1=p_mb[:, 0:128], op=OP.subtract)
        for c in range(2):
            nc.scalar.activation(sqTs[c][:], cns[c][:], AF.Square,
                                 scale=ct[:, C_SQVF + c:C_SQVF + c + 1])
        for c in range(2):
            nc.vector.tensor_tensor(out=w1ts[c][:], in0=sqTs[c][:], in1=p_eps,
                                    op=OP.add)
        for c in range(2):
            nc.scalar.activation(lnts[c][:], w1ts[c][:], AF.Ln)
        for c in range(2):
            nc.scalar.activation(rsts[c][:], lnts[c][:], AF.Exp, scale=-0.5)
        for c in range(2):
            nc.vector.tensor_tensor(out=sTbs[c][:], in0=cns[c][:], in1=rsts[c][:],
                                    op=OP.mult)
        for c in range(2):
            for g in range(2):
                hs = slice(2 * g, 2 * (g + 1))
                nc.vector.tensor_tensor(
                    out=scTs[c][:, hs, :], in0=acq[:, c, hs, :],
                    in1=sTbs[c][:].unsqueeze(1).broadcast_to((128, 2, 128)),
                    op=OP.mult)
        for c in range(2):
            for g in range(2):
                hs = slice(2 * g, 2 * (g + 1))
                nc.scalar.activation(eTs[c][:, hs, :], scTs[c][:, hs, :],
                                     AF.Exp, bias=ct[:, C_MK + c:C_MK + c + 1])
        for c in range(2):
            for g in range(2):
                hs = slice(2 * g, 2 * (g + 1))
                nc.vector.tensor_tensor(
                    out=wpres[c][:, hs, :], in0=eTs[c][:, hs, :],
                    in1=sTbs[c][:].unsqueeze(1).broadcast_to((128, 2, 128)),
                    op=OP.mult)
        for c in range(2):
            for h in range(H):
                nc.tensor.matmul(pz4[:, h:h + 1], eTs[c][:, h, :], oneb[:],
                                 start=False, stop=(c == 1 and h == H - 1),
                                 skip_group_check=True)
                uv = cb[:, B_UV + 128 * c + 32 * h:B_UV + 128 * c + 32 * (h + 1)]
                nc.tensor.matmul(p_att[:, 32 * h:32 * (h + 1)],
                                 wpres[c][:, h, :], uv,
                                 start=False, stop=(c == 1), skip_group_check=True)

        # ---------------- softmax normalize + output proj ----------------
        rz4 = cp.tile([128, H], BF16, tag="rz4", name="rz4")
        with nc.allow_low_precision(reason="softmax denom recip in bf16"):
            nc.vector.reciprocal(rz4[:], pz4)
        oa2 = cp.tile([128, H, 32], BF16, tag="oa2", name="oa2")
        nc.vector.tensor_tensor(
            out=oa2[:], in0=p_att[:].rearrange("p (h d) -> p h d", h=H),
            in1=rz4[:].unsqueeze(2).broadcast_to((128, H, 32)), op=OP.mult)
        p_oa3 = ps_sm.tile([128, 128], BF16, tag="sm", name="p_oa3")
        nc.tensor.transpose(p_oa3[:], oa2[:].rearrange("p a b -> p (a b)"),
                            identb)
        oa3 = cp.tile([128, 128], BF16, tag="oa3", name="oa3")
        nc.vector.tensor_copy(out=oa3[:], in_=p_oa3[:])
        p_wo = ps_big.tile([128, 128], F32, tag="big", name="p_wo")
        nc.tensor.matmul(p_wo[:], onebr[:], wobarb, start=True, stop=False,
                         skip_group_check=True)
        nc.tensor.matmul(p_wo[:], oa3[:], cb[:, B_WO:B_WO + 128], start=False,
                         stop=True, skip_group_check=True)
        # ---------------- final LN (already [n, e]) ----------------
        stO = sb("stO", [128, 6])
        nc.vector.bn_stats(stO[:], p_wo[:])
        agO = sb("agO", [128, 2])
        nc.vector.bn_aggr(agO[:], stO[:])
        vpe = sb("vpe", [128, 1])
        nc.vector.tensor_tensor(out=vpe[:], in0=agO[:, 1:2],
                                in1=ct[:, C_A2E:C_A2E + 1], op=OP.add)
        oob = cp.tile([128, 128], BF16, tag="oob", name="oob")
        nc.vector.tensor_copy(out=oob[:], in_=p_wo[:])
        lnv = sb("lnv", [128, 1])
        nc.scalar.activation(lnv[:], vpe[:], AF.Ln)
        r = sb("r", [128, 1])
        nc.scalar.activation(r[:], lnv[:], AF.Exp, scale=-0.5)
        # act-table prefetch: gelu-set load lands in the hn/T2/FFN-mm window
        dum = sb("dum", [1, 1], cp)
        nc.scalar.activation(dum[:], r[0:1, 0:1], AF.Gelu)
        hn = cp.tile([128, 128], BF16, tag="hn", name="hn")
        nc.vector.tensor_scalar(out=hn[:], in0=p_wo[:], scalar1=agO[:, 0:1],
                                scalar2=r[:, 0:1], op0=OP.subtract, op1=OP.mult)
        p_hT = ps_big.tile([128, 128], BF16, tag="big", name="p_hT")
        nc.tensor.transpose(p_hT[:], hn[:], identb)
        hT = cp.tile([128, 128], BF16, tag="hT", name="hT")
        nc.vector.tensor_copy(out=hT[:], in_=p_hT[:])

        # ---------------- FFN (bf16) ----------------
        gts = []
        for i, bcol in enumerate((C_B1A, C_B1B)):
            p_f1 = ps_big.tile([128, 128], F32, tag="big", name=f"p_f1{i}")
            nc.tensor.matmul(p_f1[:], cb[:, B_W1 + 128 * i:B_W1 + 128 * (i + 1)],
                             hT[:], start=True, stop=True)
            gt = cp.tile([128, 128], BF16, tag=f"gt{i}", name=f"gt{i}")
            nc.scalar.activation(gt[:], p_f1[:], AF.Gelu,
                                 bias=ct[:, bcol:bcol + 1])
            gts.append(gt)
        p_y = ps_big.tile([128, 128], F32, tag="big", name="p_y")
        nc.tensor.matmul(p_y[:], onebr[:], cvbarb, start=True, stop=False,
                         skip_group_check=True)
        nc.tensor.matmul(p_y[:], gts[0][:], cb[:, B_W2A:B_W2A + 128], start=False,
                         stop=False, skip_group_check=True)
        nc.tensor.matmul(p_y[:], gts[1][:], cb[:, B_W2B:B_W2B + 128], start=False,
                         stop=True, skip_group_check=True)
        zf = wp.tile([128, 128], BF16, tag="zf", name="zf")
        nc.vector.tensor_tensor(out=zf[:, 0:64], in0=p_y[:, 0:64],
                                in1=oob[:, 0:64], op=OP.add)
        nc.vector.tensor_tensor(out=zf[:, 64:128], in0=p_y[:, 64:128],
                                in1=oob[:, 64:128], op=OP.add)
        nc.sync.dma_start(out_ap[:, 0:64], zf[:, 0:64])
        nc.scalar.dma_start(out_ap[:, 64:128], zf[:, 64:128])


_CACHE = {}


def _restrict_act_tables():
    """Limit the act-table-load pass to two sets so every non-Gelu activation
    (copy/exp/ln/square) resolves to one table and Gelu to the other."""
    import concourse.hw_specs as hws
    import concourse.bacc as bacc_mod
    orig = hws.get_activation_tables

    def patched(arch):
        t = orig(arch)
        keep = {}
        n_good = 0
        for name, fns in t.items():
            fnames = {f.name for f in fns}
            good = ("Ln" in fnames and "Exp" in fnames) or "Gelu" in fnames
            keep[name] = fns if good else set()   # keep positions for set ids
            n_good += bool(good)
        assert n_good >= 2, f"unexpected act table sets: {list(t)}"
        return keep

    bacc_mod.get_activation_tables = patched


def _get_nc():
    if "nc" in _CACHE:
        return _CACHE["nc"]
    _restrict_act_tables()
    nc = bacc.Bacc("TRN2", target_bir_lowering=False, debug=False,
                   num_devices=NCORES)
    d = {}
    d["zin"] = nc.dram_tensor("zin", [128, FD + CE], F32,
                              kind="ExternalInput").ap()
    d["zt"] = nc.dram_tensor("zt", [128, FD], F32, kind="ExternalInput").ap()
    d["cba"] = nc.dram_tensor("cba", [128, CB + 2 * H * 128], BF16,
                              kind="ExternalInput").ap()
    out_ap = nc.dram_tensor("out", [NP, E], BF16, kind="ExternalOutput").ap()
    with tile.TileContext(nc) as tc:
        _body(tc, d, out_ap)
    nc.compile()
    _CACHE["nc"] = nc
    return nc


def _host_consts(a):
    """Weight-only constants, computed in float64 exactly as the reference."""
    fe = a["feat_emb"].astype(np.float64)
    g1 = a["g1"].astype(np.float64)
    beta1 = a["beta1"].astype(np.float64)
    g2 = a["g2"].astype(np.float64)
    beta2 = a["beta2"].astype(np.float64)
    Wq, bq = a["Wq"].astype(np.float64), a["bq"].astype(np.float64)
    Wk, bk = a["Wk"].astype(np.float64), a["bk"].astype(np.float64)
    Wv, bv = a["Wv"].astype(np.float64), a["bv"].astype(np.float64)
    Wo, bo = a["Wo"].astype(np.float64), a["bo"].astype(np.float64)
    W1, b1 = a["W1"].astype(np.float64), a["b1"].astype(np.float64)
    W2, b2 = a["W2"].astype(np.float64), a["b2"].astype(np.float64)
    al = float(np.asarray(a["alpha_res"]).reshape(-1)[0])

    mf = fe.mean(axis=1, keepdims=True)
    u = fe - mf
    vf = (u * u).mean(axis=1)                     # [256]
    sqvf = np.sqrt(vf)

    lab = a["label_token"].astype(np.float64).reshape(E)
    mL = lab.mean()
    vL = ((lab - mL) ** 2).mean()
    xl0 = (lab - mL) / np.sqrt(vL + EPS)
    dcol = xl0 * g1
    xlast = dcol + beta1                          # X_norm label row [E]

    q = xlast @ Wq + bq                           # [E]
    ug = u * g1[None, :]
    UK = ug @ Wk                                  # [256, E]
    ck = beta1 @ Wk + bk
    UV = ug @ Wv                                  # [256, E]
    cv = beta1 @ Wv + bv                          # [E]
    Klab = dcol @ Wk + ck
    vd = dcol @ Wv                                # label V row minus cv

    acol = np.zeros((FD, H))
    cp_ = np.zeros(H)
    cpp = np.zeros(H)
    for h in range(H):
        s_ = slice(DK * h, DK * (h + 1))
        acol[:, h] = UK[:, s_] @ q[s_] * ISQ
        cp_[h] = q[s_] @ ck[s_] * ISQ
        cpp[h] = q[s_] @ Klab[s_] * ISQ + np.log1p(1e-9)
    ec = np.exp(cpp - cp_)                        # label softmax weight [H]

    A = a["A_no_diag"].astype(np.float64)
    cm = np.abs(A).T
    cmax = cm.max()
    cm = cm / cmax if cmax > 1e-6 else cm + 1e-3
    np.fill_diagonal(cm, 1.0)
    mk = np.log(cm[FD, 0:FD] + 1e-9)              # label-query row vs features

    Wo2 = al * Wo
    wobar = Wo2.T @ cv + al * bo                  # [E]
    w1p = W1 * g2[:, None]                        # [E, 2E]
    b1p = beta2 @ W1 + b1                         # [2E]
    cvbar = al * b2 + xlast                       # [E]

    import ml_dtypes
    BF = ml_dtypes.bfloat16
    cearly = np.zeros((128, CE), np.float32)
    cearly[:, C_ONE1] = 1.0
    cearly[:, C_A2E] = al * al * EPS
    cearly[:, C_B1A] = b1p[0:E]
    cearly[:, C_B1B] = b1p[E:2 * E]
    np.fill_diagonal(cearly[:, C_IDENT:C_IDENT + 128], 1.0)
    cbuf = np.zeros((128, CB), BF)
    for c in range(2):
        ch = slice(128 * c, 128 * (c + 1))
        cearly[:, C_SQVF + c] = sqvf[ch]
        cearly[:, C_MK + c] = mk[ch]
        cbuf[:, B_ACOL + H * c:B_ACOL + H * (c + 1)] = acol[ch].astype(BF)
        cbuf[:, B_UV + 128 * c:B_UV + 128 * (c + 1)] = UV[ch].astype(BF)
    for h in range(H):
        cbuf[h, B_BO4 + DK * h:B_BO4 + DK * (h + 1)] = 1.0   # bo4
    cbuf[0, B_CVB:B_CVB + E] = cvbar.astype(BF)
    cbuf[:, B_W1:B_W1 + 2 * E] = w1p.astype(BF)
    cbuf[:, B_W2A:B_W2A + E] = (al * W2[0:E]).astype(BF)
    cbuf[:, B_W2B:B_W2B + E] = (al * W2[E:2 * E]).astype(BF)

    cbuf[0, B_WOBR:B_WOBR + E] = wobar.astype(BF)
    cbuf[:, B_WO:B_WO + E] = Wo2.astype(BF)
    np.fill_diagonal(cbuf[:, B_IDENT:B_IDENT + 128], 1.0)
    cbuf[0, B_ULC:B_ULC + E] = (vd * np.repeat(ec, DK)).astype(BF)
    cbuf[0, B_ECROW:B_ECROW + H] = ec.astype(BF)
    acq = np.zeros((128, 2, H, 128), BF)
    for c in range(2):
        ch = slice(128 * c, 128 * (c + 1))
        acq[:, c] = np.broadcast_to(acol[ch].astype(BF)[:, :, None],
                                    (128, H, 128))
    return cearly, cbuf, acq.reshape(128, 2 * H * 128)


def _in_maps(inputs):
    a = {k: np.asarray(v) for k, v in inputs.items()}
    cearly, cbuf, acq = _host_consts(a)
    cba = np.concatenate([cbuf, acq], axis=1)
    Z = np.asarray(a["Z"], np.float32)
    maps = []
    for c in range(NCORES):
        zc = Z[c * NP:(c + 1) * NP]
        ztc = zc.T.reshape(2, 128, NP).transpose(1, 0, 2).reshape(128, FD)
        zin = np.concatenate([zc, cearly], axis=1)
        m = {"cba": cba, "zin": np.ascontiguousarray(zin),
             "zt": np.ascontiguousarray(ztc)}
        maps.append(m)
    return maps


def run(inputs, trace=False):
    nc = _get_nc()
    res = run_bass_kernel_spmd(nc, _in_maps(inputs), core_ids=list(range(NCORES)),
                               trace=trace)
    out = np.concatenate([res.results[c]["out"] for c in range(NCORES)], axis=0)
    return out.astype(np.float32), res


def kernel(**inputs):
    out, _ = run(inputs, trace=False)
    return out


# revision 31
# speedup vs baseline: 1.0130x; 1.0130x over previous
"""Trainium2 Bass kernel for nn_CausalFeatureTransformer.

Only the label row of the reference output is needed, so the per-sample
transformer collapses to per-sample score maps plus head-wise weighted
sums.  All weight-only quantities (score columns, mask logs, UV
projections, folded affine/alpha constants) are computed on the host in
numpy; the device runs only the Z-dependent main phase:

  bn_stats on row-layout Z for per-sample mean/var, rank-1 PE matmuls
  to broadcast rows, s-chain on sqrt(vf)-prefolded transposed Z, scores
  via one double-broadcast tensor_tensor per chunk, exp with per-
  partition mask bias, attention numerators via UV-stationary matmuls
  and denominators via eT-stationary ones-matvecs (so the softmax
  reciprocal runs on a [128,4] column tile), final LN in [n,e] layout
  where mean/rstd are per-partition scalars (magic-constant Newton
  rsqrt on [128,1] columns), FFN with pre-scaled weights and rank-1
  bias matmuls.  No gpsimd ops (no library reloads); output is stored
  [e, n] and transposed on host.

Data-parallel over N: 1024 samples -> 8 cores x 128 samples.
"""
import numpy as np
from contextlib import ExitStack

import concourse.bass as bass
import concourse.tile as tile
from concourse import bacc, mybir
from concourse.bass_utils import run_bass_kernel_spmd

F32 = mybir.dt.float32
BF16 = mybir.dt.bfloat16
I32 = mybir.dt.int32
AF = mybir.ActivationFunctionType
OP = mybir.AluOpType

N, FD, E, H, DK, SEQ = 1024, 256, 128, 4, 32, 257
NCORES = 8
NP = N // NCORES
EPS = 1e-5
ISQ = float(1.0 / np.sqrt(DK))

# cearly (f32) column layout
C_ONE1 = 0        # 1.0 column
C_A2E = 1         # alpha^2*eps column
C_SQVF = 2        # sqrt(vf) chunk cols 2,3
C_MK = 4          # mask-log chunk cols 4,5
C_B1A = 6         # b1' halves cols 6,7
C_B1B = 7
C_IDENT = 8       # identity [128,128]
CE = 136
# cb (bf16) column layout
B_UV = 0          # UV chunks at 0:128, 128:256
B_BO4 = 256       # bo4 rows 0:4 at cols 256:384
B_CVB = 384       # cvbar row 0 at cols 384:512
B_W1 = 512        # diag(g2)@W1 at 512:768
B_W2A = 768       # alpha*W2[:128]
B_W2B = 896
B_ACOL = 1024     # acol chunks [H] at 1024:1028, 1028:1032
B_IDENT = 1032    # bf16 identity [128,128]
B_WO = 1160       # alpha*Wo bf16
B_WOBR = 1288     # wobar row 0
B_ULC = 1416      # ulc row 0 [1,128]
B_ECROW = 1544    # ec row 0 [1,4]
CB = 1548



def _body(tc, d, out_ap):
    nc = tc.nc
    ctx = ExitStack()
    with ctx:
        cp = ctx.enter_context(tc.tile_pool(name="cp", bufs=1))
        wp = ctx.enter_context(tc.tile_pool(name="wp", bufs=1))
        ps_att = ctx.enter_context(tc.tile_pool(name="ps_att", bufs=2, space="PSUM"))
        ps_mb = ctx.enter_context(tc.tile_pool(name="ps_mb", bufs=1, space="PSUM"))
        ps_sm = ctx.enter_context(tc.tile_pool(name="ps_sm", bufs=1, space="PSUM"))
        ps_big = ctx.enter_context(tc.tile_pool(name="ps_big", bufs=2, space="PSUM"))

        def sb(name, shape, pool=wp):
            return pool.tile(list(shape), F32, tag=name, name=name)

        # ---------------- loads ----------------
        # [z | cearly] in one transfer (both gate the front chain); zt after
        zin = sb("zin", [128, FD + CE], cp)
        nc.sync.dma_start(zin[:], d["zin"])
        z = zin[:, 0:FD]                        # row layout [n, f]
        ct = zin[:, FD:FD + CE]
        zt = sb("zt", [128, FD], cp)            # Z^T chunks [f, n]
        nc.sync.dma_start(zt[:], d["zt"])
        # big bf16 consts go out on the scalar queue, gated on zin completion,
        # so their descriptors don't round-robin-delay the zin transfer
        dumz = sb("dumz", [1, 1], cp)
        nc.scalar.copy(dumz[:], zin[0:1, 0:1])
        cba = cp.tile([128, CB + 2 * H * 128], BF16, tag="cba", name="cba")
        nc.scalar.dma_start(cba[:], d["cba"])
        cb = cba[:, 0:CB]
        acq = cba[:, CB:CB + 2 * H * 128].rearrange("p (a h b) -> p a h b",
                                                    a=2, h=H)

        one1 = ct[:, C_ONE1:C_ONE1 + 1]
        ident = ct[:, C_IDENT:C_IDENT + 128]
        identb = cb[:, B_IDENT:B_IDENT + 128]
        bo4b = cb[0:4, B_BO4:B_BO4 + 128]
        cvbarb = cb[0:1, B_CVB:B_CVB + 128]
        wobarb = cb[0:1, B_WOBR:B_WOBR + 128]
        ecrowb = cb[0:1, B_ECROW:B_ECROW + H]

        ones1 = sb("ones1", [1, 128], cp)
        nc.vector.memset(ones1[:], 1.0)
        oneb = cp.tile([128, 1], BF16, tag="oneb", name="oneb")
        nc.vector.memset(oneb[:], 1.0)
        onebr = cp.tile([1, 128], BF16, tag="onebr", name="onebr")
        nc.vector.memset(onebr[:], 1.0)

        # one bank: [0:128] m-bcast, [128:256] eps-bcast, [384:388] pz4
        p_mb = ps_mb.tile([128, 388], F32, tag="mb", name="p_mb")
        pz4 = p_mb[:, 384:388]
        p_att = ps_att.tile([128, 128], F32, tag="att", name="p_att")

        # ---------------- Z stats (row layout) + row broadcasts ----------------
        stZ = sb("stZ", [128, 6])
        nc.vector.bn_stats(stZ[:], z[:])
        me = sb("me", [128, 2])
        nc.vector.bn_aggr(me[:], stZ[:])
        nc.vector.tensor_scalar(out=me[:, 1:2], in0=me[:, 1:2], scalar1=EPS,
                                scalar2=EPS * EPS, op0=OP.mult, op1=OP.add)
        p_me = ps_sm.tile([1, 256], F32, tag="sm", name="p_me")
        nc.tensor.transpose(p_me[0:1, 0:128], me[:, 0:1], ident)
        nc.tensor.transpose(p_me[0:1, 128:256], me[:, 1:2], ident)
        mer = cp.tile([1, 256], BF16, tag="mer", name="mer")
        nc.vector.tensor_copy(out=mer[:, 0:128], in_=p_me[0:1, 0:128])
        nc.tensor.matmul(p_mb[:, 0:128], onebr[:], mer[:, 0:128], start=True,
                         stop=True, skip_group_check=True)
        nc.vector.tensor_copy(out=mer[:, 128:256], in_=p_me[0:1, 128:256])
        nc.tensor.matmul(p_mb[:, 128:256], onebr[:], mer[:, 128:256],
                         start=True, stop=True, skip_group_check=True)
        p_eps = p_mb[:, 128:256]
        # label-token rank-1 PSUM inits (after p_mb so it wins the PE queue)
        nc.tensor.matmul(pz4, onebr[:], ecrowb, start=True, stop=False,
                         skip_group_check=True)
        nc.tensor.matmul(p_att[:], onebr[:], cb[0:1, B_ULC:B_ULC + 128],
                         start=True, stop=False, skip_group_check=True)

        # ------ s, scores, exp, attention matmuls ------
        # two chunks, emitted stage-interleaved so chunk 1's vector stages
        # overlap chunk 0's scalar stages in the in-order engine queues
        cns, sqTs, w1ts, lnts, rsts, sTbs, scTs, eTs, wpres = \
            [], [], [], [], [], [], [], [], []
        for c in range(2):
            cns.append(sb(f"cn{c}", [128, 128]))
            sqTs.append(sb(f"sqT{c}", [128, 128]))
            w1ts.append(sb(f"w1t{c}", [128, 128]))
            lnts.append(sb(f"lnt{c}", [128, 128]))
            rsts.append(sb(f"rst{c}", [128, 128]))
            sTbs.append(wp.tile([128, 128], BF16, tag=f"sTb{c}", name=f"sTb{c}"))
            scTs.append(wp.tile([128, H, 128], BF16, tag=f"scT{c}",
                                name=f"scT{c}"))
            eTs.append(wp.tile([128, H, 128], BF16, tag=f"eT{c}", name=f"eT{c}"))
            wpres.append(wp.tile([128, H, 128], BF16, tag=f"wpre{c}",
                                 name=f"wpre{c}"))
        for c in range(2):
            nc.vector.tensor_tensor(out=cns[c][:],
                                    in0=zt[:, 128 * c:128 * (c + 1)],
                                    in1=p_mb[:, 0:128], op=OP.subtract)
        for c in range(2):
            nc.scalar.activation(sqTs[c][:], cns[c][:], AF.Square,
                                 scale=ct[:, C_SQVF + c:C_SQVF + c + 1])
        for c in range(2):
            nc.vector.tensor_tensor(out=w1ts[c][:], in0=sqTs[c][:], in1=p_eps,
                                    op=OP.add)
        for c in range(2):
            nc.scalar.activation(lnts[c][:], w1ts[c][:], AF.Ln)
        for c in range(2):
            nc.scalar.activation(rsts[c][:], lnts[c][:], AF.Exp, scale=-0.5)
        for c in range(2):
            nc.vector.tensor_tensor(out=sTbs[c][:], in0=cns[c][:], in1=rsts[c][:],
                                    op=OP.mult)
        for c in range(2):
            nc.vector.tensor_tensor(
                out=scTs[c][:], in0=acq[:, c],
                in1=sTbs[c][:].unsqueeze(1).broadcast_to((128, H, 128)),
                op=OP.mult)
        for c in range(2):
            nc.scalar.activation(eTs[c][:], scTs[c][:], AF.Exp,
                                 bias=ct[:, C_MK + c:C_MK + c + 1])
        for c in range(2):
            nc.vector.tensor_tensor(
                out=wpres[c][:], in0=eTs[c][:],
                in1=sTbs[c][:].unsqueeze(1).broadcast_to((128, H, 128)),
                op=OP.mult)
        for c in range(2):
            for h in range(H):
                nc.tensor.matmul(pz4[:, h:h + 1], eTs[c][:, h, :], oneb[:],
                                 start=False, stop=(c == 1 and h == H - 1),
                                 skip_group_check=True)
                uv = cb[:, B_UV + 128 * c + 32 * h:B_UV + 128 * c + 32 * (h + 1)]
                nc.tensor.matmul(p_att[:, 32 * h:32 * (h + 1)],
                                 wpres[c][:, h, :], uv,
                                 start=False, stop=(c == 1), skip_group_check=True)

        # ---------------- softmax normalize + output proj ----------------
        rz4 = cp.tile([128, H], BF16, tag="rz4", name="rz4")
        with nc.allow_low_precision(reason="softmax denom recip in bf16"):
            nc.vector.reciprocal(rz4[:], pz4)
        oa2 = cp.tile([128, H, 32], BF16, tag="oa2", name="oa2")
        nc.vector.tensor_tensor(
            out=oa2[:], in0=p_att[:].rearrange("p (h d) -> p h d", h=H),
            in1=rz4[:].unsqueeze(2).broadcast_to((128, H, 32)), op=OP.mult)
        p_oa3 = ps_sm.tile([128, 128], BF16, tag="sm", name="p_oa3")
        nc.tensor.transpose(p_oa3[:], oa2[:].rearrange("p a b -> p (a b)"),
                            identb)
        oa3 = cp.tile([128, 128], BF16, tag="oa3", name="oa3")
        nc.vector.tensor_copy(out=oa3[:], in_=p_oa3[:])
        p_wo = ps_big.tile([128, 128], F32, tag="big", name="p_wo")
        nc.tensor.matmul(p_wo[:], onebr[:], wobarb, start=True, stop=False,
                         skip_group_check=True)
        nc.tensor.matmul(p_wo[:], oa3[:], cb[:, B_WO:B_WO + 128], start=False,
                         stop=True, skip_group_check=True)
        # ---------------- final LN (already [n, e]) ----------------
        stO = sb("stO", [128, 6])
        nc.vector.bn_stats(stO[:], p_wo[:])
        agO = sb("agO", [128, 2])
        nc.vector.bn_aggr(agO[:], stO[:])
        vpe = sb("vpe", [128, 1])
        nc.vector.tensor_tensor(out=vpe[:], in0=agO[:, 1:2],
                                in1=ct[:, C_A2E:C_A2E + 1], op=OP.add)
        oob = cp.tile([128, 128], BF16, tag="oob", name="oob")
        nc.vector.tensor_copy(out=oob[:], in_=p_wo[:])
        lnv = sb("lnv", [128, 1])
        nc.scalar.activation(lnv[:], vpe[:], AF.Ln)
        r = sb("r", [128, 1])
        nc.scalar.activation(r[:], lnv[:], AF.Exp, scale=-0.5)
        # act-table prefetch: gelu-set load lands in the hn/T2/FFN-mm window
        dum = sb("dum", [1, 1], cp)
        nc.scalar.activation(dum[:], r[0:1, 0:1], AF.Gelu)
        hn = cp.tile([128, 128], BF16, tag="hn", name="hn")
        nc.vector.tensor_scalar(out=hn[:], in0=p_wo[:], scalar1=agO[:, 0:1],
                                scalar2=r[:, 0:1], op0=OP.subtract, op1=OP.mult)
        p_hT = ps_big.tile([128, 128], BF16, tag="big", name="p_hT")
        nc.tensor.transpose(p_hT[:], hn[:], identb)
        hT = cp.tile([128, 128], BF16, tag="hT", name="hT")
        nc.vector.tensor_copy(out=hT[:], in_=p_hT[:])

        # ---------------- FFN (bf16) ----------------
        gts = []
        for i, bcol in enumerate((C_B1A, C_B1B)):
            p_f1 = ps_big.tile([128, 128], F32, tag="big", name=f"p_f1{i}")
            nc.tensor.matmul(p_f1[:], cb[:, B_W1 + 128 * i:B_W1 + 128 * (i + 1)],
                             hT[:], start=True, stop=True)
            gt = cp.tile([128, 128], BF16, tag=f"gt{i}", name=f"gt{i}")
            nc.scalar.activation(gt[:], p_f1[:], AF.Gelu,
                                 bias=ct[:, bcol:bcol + 1])
            gts.append(gt)
        p_y = ps_big.tile([128, 128], F32, tag="big", name="p_y")
        nc.tensor.matmul(p_y[:], onebr[:], cvbarb, start=True, stop=False,
                         skip_group_check=True)
        nc.tensor.matmul(p_y[:], gts[0][:], cb[:, B_W2A:B_W2A + 128], start=False,
                         stop=False, skip_group_check=True)
        nc.tensor.matmul(p_y[:], gts[1][:], cb[:, B_W2B:B_W2B + 128], start=False,
                         stop=True, skip_group_check=True)
        zf = wp.tile([128, 128], BF16, tag="zf", name="zf")
        nc.vector.tensor_tensor(out=zf[:, 0:64], in0=p_y[:, 0:64],
                                in1=oob[:, 0:64], op=OP.add)
        nc.vector.tensor_tensor(out=zf[:, 64:128], in0=p_y[:, 64:128],
                                in1=oob[:, 64:128], op=OP.add)
        nc.sync.dma_start(out_ap[:, 0:64], zf[:, 0:64])
        nc.scalar.dma_start(out_ap[:, 64:128], zf[:, 64:128])


_CACHE = {}


def _restrict_act_tables():
    """Limit the act-table-load pass to two sets so every non-Gelu activation
    (copy/exp/ln/square) resolves to one table and Gelu to the other."""
    import concourse.hw_specs as hws
    import concourse.bacc as bacc_mod
    orig = hws.get_activation_tables

    def patched(arch):
        t = orig(arch)
        keep = {}
        n_good = 0
        for name, fns in t.items():
            fnames = {f.name for f in fns}
            good = ("Ln" in fnames and "Exp" in fnames) or "Gelu" in fnames
            keep[name] = fns if good else set()   # keep positions for set ids
            n_good += bool(good)
        assert n_good >= 2, f"unexpected act table sets: {list(t)}"
        return keep

    bacc_mod.get_activation_tables = patched


def _get_nc():
    if "nc" in _CACHE:
        return _CACHE["nc"]
    _restrict_act_tables()
    nc = bacc.Bacc("TRN2", target_bir_lowering=False, debug=False,
                   num_devices=NCORES)
    d = {}
    d["zin"] = nc.dram_tensor("zin", [128, FD + CE], F32,
                              kind="ExternalInput").ap()
    d["zt"] = nc.dram_tensor("zt", [128, FD], F32, kind="ExternalInput").ap()
    d["cba"] = nc.dram_tensor("cba", [128, CB + 2 * H * 128], BF16,
                              kind="ExternalInput").ap()
    out_ap = nc.dram_tensor("out", [NP, E], BF16, kind="ExternalOutput").ap()
    with tile.TileContext(nc) as tc:
        _body(tc, d, out_ap)
    nc.compile()
    _CACHE["nc"] = nc
    return nc


def _host_consts(a):
    """Weight-only constants, computed in float64 exactly as the reference."""
    fe = a["feat_emb"].astype(np.float64)
    g1 = a["g1"].astype(np.float64)
    beta1 = a["beta1"].astype(np.float64)
    g2 = a["g2"].astype(np.float64)
    beta2 = a["beta2"].astype(np.float64)
    Wq, bq = a["Wq"].astype(np.float64), a["bq"].astype(np.float64)
    Wk, bk = a["Wk"].astype(np.float64), a["bk"].astype(np.float64)
    Wv, bv = a["Wv"].astype(np.float64), a["bv"].astype(np.float64)
    Wo, bo = a["Wo"].astype(np.float64), a["bo"].astype(np.float64)
    W1, b1 = a["W1"].astype(np.float64), a["b1"].astype(np.float64)
    W2, b2 = a["W2"].astype(np.float64), a["b2"].astype(np.float64)
    al = float(np.asarray(a["alpha_res"]).reshape(-1)[0])

    mf = fe.mean(axis=1, keepdims=True)
    u = fe - mf
    vf = (u * u).mean(axis=1)                     # [256]
    sqvf = np.sqrt(vf)

    lab = a["label_token"].astype(np.float64).reshape(E)
    mL = lab.mean()
    vL = ((lab - mL) ** 2).mean()
    xl0 = (lab - mL) / np.sqrt(vL + EPS)
    dcol = xl0 * g1
    xlast = dcol + beta1                          # X_norm label row [E]

    q = xlast @ Wq + bq                           # [E]
    ug = u * g1[None, :]
    UK = ug @ Wk                                  # [256, E]
    ck = beta1 @ Wk + bk
    UV = ug @ Wv                                  # [256, E]
    cv = beta1 @ Wv + bv                          # [E]
    Klab = dcol @ Wk + ck
    vd = dcol @ Wv                                # label V row minus cv

    acol = np.zeros((FD, H))
    cp_ = np.zeros(H)
    cpp = np.zeros(H)
    for h in range(H):
        s_ = slice(DK * h, DK * (h + 1))
        acol[:, h] = UK[:, s_] @ q[s_] * ISQ
        cp_[h] = q[s_] @ ck[s_] * ISQ
        cpp[h] = q[s_] @ Klab[s_] * ISQ + np.log1p(1e-9)
    ec = np.exp(cpp - cp_)                        # label softmax weight [H]

    A = a["A_no_diag"].astype(np.float64)
    cm = np.abs(A).T
    cmax = cm.max()
    cm = cm / cmax if cmax > 1e-6 else cm + 1e-3
    np.fill_diagonal(cm, 1.0)
    mk = np.log(cm[FD, 0:FD] + 1e-9)              # label-query row vs features

    Wo2 = al * Wo
    wobar = Wo2.T @ cv + al * bo                  # [E]
    w1p = W1 * g2[:, None]                        # [E, 2E]
    b1p = beta2 @ W1 + b1                         # [2E]
    cvbar = al * b2 + xlast                       # [E]

    import ml_dtypes
    BF = ml_dtypes.bfloat16
    cearly = np.zeros((128, CE), np.float32)
    cearly[:, C_ONE1] = 1.0
    cearly[:, C_A2E] = al * al * EPS
    cearly[:, C_B1A] = b1p[0:E]
    cearly[:, C_B1B] = b1p[E:2 * E]
    np.fill_diagonal(cearly[:, C_IDENT:C_IDENT + 128], 1.0)
    cbuf = np.zeros((128, CB), BF)
    for c in range(2):
        ch = slice(128 * c, 128 * (c + 1))
        cearly[:, C_SQVF + c] = sqvf[ch]
        cearly[:, C_MK + c] = mk[ch]
        cbuf[:, B_ACOL + H * c:B_ACOL + H * (c + 1)] = acol[ch].astype(BF)
        cbuf[:, B_UV + 128 * c:B_UV + 128 * (c + 1)] = UV[ch].astype(BF)
    for h in range(H):
        cbuf[h, B_BO4 + DK * h:B_BO4 + DK * (h + 1)] = 1.0   # bo4
    cbuf[0, B_CVB:B_CVB + E] = cvbar.astype(BF)
    cbuf[:, B_W1:B_W1 + 2 * E] = w1p.astype(BF)
    cbuf[:, B_W2A:B_W2A + E] = (al * W2[0:E]).astype(BF)
    cbuf[:, B_W2B:B_W2B + E] = (al * W2[E:2 * E]).astype(BF)

    cbuf[0, B_WOBR:B_WOBR + E] = wobar.astype(BF)
    cbuf[:, B_WO:B_WO + E] = Wo2.astype(BF)
    np.fill_diagonal(cbuf[:, B_IDENT:B_IDENT + 128], 1.0)
    cbuf[0, B_ULC:B_ULC + E] = (vd * np.repeat(ec, DK)).astype(BF)
    cbuf[0, B_ECROW:B_ECROW + H] = ec.astype(BF)
    acq = np.zeros((128, 2, H, 128), BF)
    for c in range(2):
        ch = slice(128 * c, 128 * (c + 1))
        acq[:, c] = np.broadcast_to(acol[ch].astype(BF)[:, :, None],
                                    (128, H, 128))
    return cearly, cbuf, acq.reshape(128, 2 * H * 128)


def _in_maps(inputs):
    a = {k: np.asarray(v) for k, v in inputs.items()}
    cearly, cbuf, acq = _host_consts(a)
    cba = np.concatenate([cbuf, acq], axis=1)
    Z = np.asarray(a["Z"], np.float32)
    maps = []
    for c in range(NCORES):
        zc = Z[c * NP:(c + 1) * NP]
        ztc = zc.T.reshape(2, 128, NP).transpose(1, 0, 2).reshape(128, FD)
        zin = np.concatenate([zc, cearly], axis=1)
        m = {"cba": cba, "zin": np.ascontiguousarray(zin),
             "zt": np.ascontiguousarray(ztc)}
        maps.append(m)
    return maps


def run(inputs, trace=False):
    nc = _get_nc()
    res = run_bass_kernel_spmd(nc, _in_maps(inputs), core_ids=list(range(NCORES)),
                               trace=trace)
    out = np.concatenate([res.results[c]["out"] for c in range(NCORES)], axis=0)
    return out.astype(np.float32), res


def kernel(**inputs):
    out, _ = run(inputs, trace=False)
    return out
